# revision 16
# baseline (speedup 1.0000x reference)
"""Trainium2 Bass kernel for a species-routed MoE readout layer.

Math (see problem reference): per atom x [512]:
  u = silu(emb[species]); scores = softmax(u @ Wr.T)  -> top-2 sparse gates
  out = sum_e gate_e * (W2_e @ silu(W1_e @ x + b1_e) + b2_e)
      + sum_s (W2_s @ silu(W1_s @ x + b1_s) + b2_s)          # 2 shared experts

The router depends only on species_idx (64 species), so the per-atom top-2
gates collapse to a host-computed 64x6 lookup table. Atoms are grouped by
their top-2 expert pair and each group is split evenly across the 8 cores so
the single SPMD program sees the same tile->active-expert pattern on every
core; interior tiles then only compute 2 routed + 2 shared expert MLPs.

Precision split (validated on host, fp8_sim.py): the routed contributions
are gated by ~1/6-scale softmax scores while the shared experts carry full
weight, so the two ROUTED expert MLPs run in fp8-e4m3 with DoubleRow matmuls
(2 fp8 weights/PE cell -> 2x matmul FLOP rate, contraction 256/instruction)
while the two SHARED experts stay bf16. Routed weights are scaled x16 into
e4m3's normal range; the x16 on W1 is undone by the ScalarE Silu's scale
operand, the x16 on W2 rides the PSUM and is matched by scaling the shared
W2 (bf16, exact) and b2 by 16 too; the host divides the output by 16.
Measured max-rel error ~9e-3 vs the 2e-2 gate (bf16 baseline: 3.4e-3).

Device side (per core, per <=512-atom variable-size tile, tiles aligned to
expert-pair segment boundaries):
  routed e: hps = sum_k DoubleRowMM(16*W1T8[e], x8) ;
            h = Silu(hps/16 + b1) (ScalarE) ;
            hpm8[j] = (h + alpha_e) * gate  (DVE STT, fp8 out; alpha_e =
            lstsq(W2_e, b2_e) folds the gated b2 into the gate multiply) ;
            outps += DoubleRowMM(16*W2T8[e], hpm8 pair)
  shared s: bf16 matmuls as before, W2 pre-scaled x16.
Shared-expert b2 sum (x16) is added on the PSUM->SBUF copy (DVE).
A short PE spin + dummy activation at kernel start warms the HAM clock
gate and the ACT table while the first DMAs are in flight; per-expert
weight tiles stream in first-use order behind tile 0's inputs.
"""

import numpy as np
import ml_dtypes

import concourse.bass as bass
import concourse.mybir as mybir
from concourse import bacc, tile
from concourse.bass_utils import run_bass_kernel_spmd

BF16 = mybir.dt.bfloat16
FP8 = mybir.dt.float8e4
F32 = mybir.dt.float32
BF16_NP = ml_dtypes.bfloat16
FP8_NP = ml_dtypes.float8_e4m3

N_CORES = 8
N_ATOMS = 100000
IN_F = 512
HID = 512
OUT_F = 256
N_ROUTED = 6
N_SHARED = 2
N_EXP = N_ROUTED + N_SHARED
TOPK = 2
TILE_N = 512  # atoms per tile = one PSUM bank = max matmul moving dim
KC = IN_F // 128   # 4 contraction chunks for mm1
MC = HID // 128    # 4 hid chunks
OC = OUT_F // 128  # 2 out chunks
WS = 16.0  # fp8 routed weight scale (power of 2; undone via Silu scale/host)

SPARSE = True  # compute only active routed experts per tile
# CoreSim has no Silu: decompose as x*sigmoid(x) (bias folded into PSUM via a
# rank-1 matmul). HW path uses native ScalarE Silu with the f32 bias operand.
SILU_DECOMP = False
DR = mybir.MatmulPerfMode.DoubleRow


def _silu(x):
    return x / (1.0 + np.exp(-x))


def _router_table(emb, W_router):
    """[64, 6] sparse top-2 gate table + per-species expert pair."""
    u = _silu(emb.astype(np.float32))
    logits = u @ W_router.astype(np.float32).T
    m = logits.max(axis=-1, keepdims=True)
    e = np.exp(logits - m)
    scores = e / e.sum(axis=-1, keepdims=True)
    order = np.argsort(-scores, axis=-1, kind="stable")
    top2 = order[:, :TOPK]
    wt = np.zeros_like(scores)
    rows = np.arange(scores.shape[0])[:, None]
    wt[rows, top2] = scores[rows, top2]
    return wt, top2


def _plan_sharding(species_idx, top2):
    """Group atoms by top-2 expert pair, split each group evenly over cores.

    Returns (idx_cores [N_CORES, NL] int64 with -1 padding, tiles) where
    tiles is a list of (n_atoms, segments) per full-width tile and segments
    is a tuple of (col_offset, n_cols, routed_experts); identical for every
    core by construction. Tiles are exactly TILE_N wide (except the last):
    the shared experts run on the full tile while the routed experts run on
    per-segment column ranges, so merging expert-pair segments into full
    tiles costs no extra routed compute.
    """
    n = species_idx.shape[0]
    if not SPARSE:
        assert n % N_CORES == 0
        nl = n // N_CORES
        idx_cores = np.arange(n, dtype=np.int64).reshape(N_CORES, nl)
        tiles = []
        for t0 in range(0, nl, TILE_N):
            w = min(TILE_N, nl - t0)
            tiles.append((w, ((0, w, tuple(range(N_ROUTED))),)))
        return idx_cores, tiles

    MIN_TILE = 64  # merge segments smaller than this into their neighbor

    pair_of_species = [tuple(sorted(top2[s])) for s in range(top2.shape[0])]
    pairs = sorted(set(pair_of_species))
    pair_id_of_species = np.array(
        [pairs.index(p) for p in pair_of_species], dtype=np.int64
    )
    atom_pair = pair_id_of_species[species_idx]

    seg_lens = []       # per-group per-core segment length
    seg_experts = []
    group_idx = []      # per-group atom index arrays
    for g, p in enumerate(pairs):
        idx_g = np.nonzero(atom_pair == g)[0]
        if idx_g.size == 0:
            continue
        L = -(-idx_g.size // N_CORES)  # ceil
        seg_lens.append(L)
        seg_experts.append(tuple(int(x) for x in p))
        group_idx.append(idx_g)

    # largest group first: deep pipeline while the clock warms, short tail
    order = np.argsort([-L for L in seg_lens], kind="stable")
    seg_lens = [seg_lens[i] for i in order]
    seg_experts = [seg_experts[i] for i in order]
    group_idx = [group_idx[i] for i in order]

    nl = sum(seg_lens)
    idx_cores = np.full((N_CORES, nl), -1, dtype=np.int64)
    off = 0
    for L, idx_g in zip(seg_lens, group_idx):
        for c in range(N_CORES):
            part = idx_g[c * L : (c + 1) * L]
            idx_cores[c, off : off + part.size] = part
        off += L

    # Merge tiny segments into their neighbor (their experts union), then
    # pack segments into exactly-TILE_N tiles, splitting segments at tile
    # cuts.
    segs = []  # (length, experts)
    pend_n, pend_e = 0, set()
    for L, p in zip(seg_lens, seg_experts):
        pend_n += L
        pend_e.update(p)
        if pend_n < MIN_TILE:
            continue
        segs.append((pend_n, tuple(sorted(pend_e))))
        pend_n, pend_e = 0, set()
    if pend_n:
        if segs:
            n0, e0 = segs.pop()
            pend_n += n0
            pend_e.update(e0)
        segs.append((pend_n, tuple(sorted(pend_e))))

    tiles = []
    cur, cur_n = [], 0
    for L, ee in segs:
        while L > 0:
            take = min(L, TILE_N - cur_n)
            if cur and cur[-1][2] == ee:
                cur[-1] = (cur[-1][0], cur[-1][1] + take, ee)
            else:
                cur.append((cur_n, take, ee))
            cur_n += take
            L -= take
            if cur_n == TILE_N:
                tiles.append((cur_n, tuple(cur)))
                cur, cur_n = [], 0
    if cur_n:
        tiles.append((cur_n, tuple(cur)))
    assert sum(t[0] for t in tiles) == nl
    return idx_cores, tiles


def _build_program(nl, tiles):
    nc = bacc.Bacc("TRN2", target_bir_lowering=False, debug=False)

    xT_d = nc.declare_dram_parameter("xT", [IN_F, nl], BF16, isOutput=False)
    x8_d = nc.declare_dram_parameter("x8", [IN_F, nl], FP8, isOutput=False)
    w6_d = nc.declare_dram_parameter("w6", [N_ROUTED, nl], BF16, isOutput=False)
    # shared expert weights, bf16 (W2 pre-scaled x16 on host)
    w1t_d = nc.declare_dram_parameter(
        "w1t", [N_SHARED, IN_F, HID], BF16, isOutput=False
    )
    w2t_d = nc.declare_dram_parameter(
        "w2t", [N_SHARED, HID, OUT_F], BF16, isOutput=False
    )
    # routed expert weights, fp8 e4m3, scaled x16
    w1t8_d = nc.declare_dram_parameter(
        "w1t8", [N_ROUTED, IN_F, HID], FP8, isOutput=False
    )
    w2t8_d = nc.declare_dram_parameter(
        "w2t8", [N_ROUTED, HID, OUT_F], FP8, isOutput=False
    )
    b1_d = nc.declare_dram_parameter("b1", [128, N_EXP * MC], F32, isOutput=False)
    b1r_d = nc.declare_dram_parameter("b1r", [1, N_EXP * HID], BF16, isOutput=False)
    # alpha[e] solves W2_e @ alpha_e = b2_e (host lstsq), so the gated b2
    # rides the gate multiply: W2_e @ (w*(h+alpha)) = w*(W2_e h) + w*b2_e
    alpha_d = nc.declare_dram_parameter(
        "alpha", [128, N_ROUTED * MC], F32, isOutput=False
    )
    b2s_d = nc.declare_dram_parameter("b2s", [128, OC], F32, isOutput=False)
    outT_d = nc.declare_dram_parameter("outT", [OUT_F, nl], F32, isOutput=True)

    with tile.TileContext(nc) as tc:
        with (
            tc.tile_pool(name="consts", bufs=1) as consts,
            tc.tile_pool(name="xp", bufs=3) as xp,
            tc.tile_pool(name="x8p", bufs=3) as x8p,
            tc.tile_pool(name="w6p", bufs=2) as w6p,
            tc.tile_pool(name="wbcp", bufs=10) as wbcp,
            tc.tile_pool(name="hps", bufs=4, space="PSUM") as hpsp,
            tc.tile_pool(name="hp", bufs=8) as hp_pool,
            tc.tile_pool(name="hpm8", bufs=4) as hpm8_pool,
            tc.tile_pool(name="ops", bufs=4, space="PSUM") as outps_pool,
            tc.tile_pool(name="osb", bufs=4) as osb_pool,
        ):
            # ---- constants / weights preload ----
            b1_sb = consts.tile([128, N_EXP * MC], F32, name="b1_sb")
            alpha_sb = consts.tile([128, N_ROUTED * MC], F32, name="alpha_sb")
            b2s_sb = consts.tile([128, OC], F32, name="b2s_sb")
            ones_sb = consts.tile([1, 128], BF16, name="ones_sb")
            b1row_sb = consts.tile([1, N_EXP * HID], BF16, name="b1row_sb")
            ones_row = consts.tile([1, TILE_N], BF16, name="ones_row")

            nc.sync.dma_start(b1_sb[:], b1_d[:])
            nc.sync.dma_start(alpha_sb[:], alpha_d[:])
            nc.sync.dma_start(b2s_sb[:], b2s_d[:])
            nc.sync.dma_start(b1row_sb[:], b1r_d[:])
            nc.vector.memset(ones_sb[:], 1.0)
            nc.vector.memset(ones_row[:], 1.0)

            # Per-expert weight tiles, loaded in first-use order. Tile-granular
            # deps let tile 0's matmuls start after the first expert arrives.
            # every tile computes the shared experts first (their mm1->silu->
            # mm2 chain is shortest, and the full-width PSUM start group must
            # come first), so their weights load first
            eorder = [N_ROUTED + s for s in range(N_SHARED)]
            for _, segments in tiles:
                for _, _, ee in segments:
                    for e in ee:
                        if e not in eorder:
                            eorder.append(e)
                if len(eorder) == N_EXP:
                    break
            for e in range(N_EXP):
                if e not in eorder:
                    eorder.append(e)

            # one big strided DMA per expert per matrix (queue-issue cost is
            # per-op, transfers run on the parallel DMA engines)
            w1t_v = w1t_d.rearrange("e (k p) h -> e p k h", p=128)
            w2t_v = w2t_d.rearrange("e (m p) o -> e p m o", p=128)
            w1t8_v = w1t8_d.rearrange("e (k p) h -> e p k h", p=128)
            w2t8_v = w2t8_d.rearrange("e (m p) o -> e p m o", p=128)
            w1_sb = {}
            w2_sb = {}

            def load_expert_weights(e):
                if e < N_ROUTED:
                    w1_sb[e] = consts.tile([128, KC, HID], FP8, name=f"w1e{e}")
                    w2_sb[e] = consts.tile([128, MC, OUT_F], FP8, name=f"w2e{e}")
                    nc.sync.dma_start(w1_sb[e][:], w1t8_v[e])
                    nc.sync.dma_start(w2_sb[e][:], w2t8_v[e])
                else:
                    s = e - N_ROUTED
                    w1_sb[e] = consts.tile([128, KC, HID], BF16, name=f"w1e{e}")
                    w2_sb[e] = consts.tile([128, MC, OUT_F], BF16, name=f"w2e{e}")
                    nc.sync.dma_start(w1_sb[e][:], w1t_v[s])
                    nc.sync.dma_start(w2_sb[e][:], w2t_v[s])

            load_expert_weights(eorder[0])

            # Warm the PE HAM clock gate (cold = 1.2 GHz until ~3.4us of
            # sustained activity) and the ScalarE activation table while the
            # first input DMAs are in flight.
            warm_sb = consts.tile([128, 1], F32, name="warm_sb")
            for _ in range(12):
                warm_ps = hpsp.tile([128, TILE_N], F32, name="warm_ps", tag="hps")
                nc.tensor.matmul(
                    warm_ps[:, :], ones_sb[:, :], ones_row[0:1, :],
                    start=True, stop=True,
                )
            nc.scalar.activation(
                warm_sb[:, :], b1_sb[:, 0:1],
                mybir.ActivationFunctionType.Sigmoid
                if SILU_DECOMP
                else mybir.ActivationFunctionType.Silu,
            )

            # ---- main loop over atom tiles ----
            xT_v = xT_d.rearrange("(k p) a -> p k a", p=128)
            x8_v = x8_d.rearrange("(k p) a -> p k a", p=128)
            outT_v = outT_d.rearrange("(c p) a -> p c a", p=128)
            a0 = 0
            for t, (n, segments) in enumerate(tiles):
                x_sb = xp.tile([128, KC, TILE_N], BF16, name="x_sb", tag="x")
                x8_sb = x8p.tile([128, KC, TILE_N], FP8, name="x8_sb", tag="x8")
                w6row = w6p.tile([1, N_ROUTED, TILE_N], BF16, name="w6row", tag="w6r")
                nc.sync.dma_start(x_sb[:, :, :n], xT_v[:, :, a0 : a0 + n])
                nc.sync.dma_start(x8_sb[:, :, :n], x8_v[:, :, a0 : a0 + n])
                nc.sync.dma_start(w6row[0:1, :, :n], w6_d[:, a0 : a0 + n])

                if t == 0:
                    # stream the remaining experts' weights behind tile 0's
                    # inputs; tile 0's compute covers the transfer time
                    for e in eorder[1:]:
                        load_expert_weights(e)

                # per-atom gates broadcast across 128 partitions (GPSIMD,
                # keeps PE free); one broadcast per (segment, expert)
                wsb = {}
                for gi, (off, sn, ee) in enumerate(segments):
                    for e in ee:
                        wsb_e = wbcp.tile(
                            [128, TILE_N], BF16, name="wsb", tag="wbc"
                        )
                        nc.gpsimd.partition_broadcast(
                            wsb_e[:, :sn], w6row[0:1, e, off : off + sn]
                        )
                        wsb[gi, e] = wsb_e

                # output accumulators (PSUM values carry a x16 scale)
                outps = [
                    outps_pool.tile([128, TILE_N], F32, name="ops", tag="ops")
                    for _ in range(OC)
                ]

                def silu_to(h_ap, hps_ap, e, m, scale):
                    w = h_ap.shape[-1]
                    if SILU_DECOMP:
                        boff = e * HID + m * 128
                        nc.tensor.matmul(
                            hps_ap,
                            b1row_sb[0:1, boff : boff + 128],
                            ones_row[0:1, :w],
                            start=False, stop=True,
                        )
                        s_sb = hp_pool.tile(
                            [128, TILE_N], BF16, name="s_sb", tag="s"
                        )
                        nc.scalar.activation(
                            s_sb[:, :w], hps_ap,
                            mybir.ActivationFunctionType.Sigmoid,
                            scale=scale,
                        )
                        nc.vector.scalar_tensor_tensor(
                            h_ap, hps_ap, scale, s_sb[:, :w],
                            mybir.AluOpType.mult, mybir.AluOpType.mult,
                        )
                    else:
                        nc.scalar.activation(
                            h_ap, hps_ap,
                            mybir.ActivationFunctionType.Silu,
                            bias=b1_sb[:, e * MC + m : e * MC + m + 1],
                            scale=scale,
                        )

                def emit_routed(seg_start, stop_last):
                    # fp8 e4m3 DoubleRow experts, per column segment. With
                    # seg_start, the first mm2 group of each segment clears
                    # its PSUM subrange (per-element has_written).
                    for gi, (off, sn, ee) in enumerate(segments):
                        last_seg = gi == len(segments) - 1
                        for ei, e in enumerate(ee):
                            for kk in range(MC // 2):  # m-chunk pairs
                                hpm8 = hpm8_pool.tile(
                                    [128, 2, TILE_N], FP8,
                                    name="hpm8", tag="hpm8"
                                )
                                for j in range(2):
                                    m = 2 * kk + j
                                    hps = hpsp.tile(
                                        [128, TILE_N], F32,
                                        name="hps", tag="hps"
                                    )
                                    for k in range(KC // 2):
                                        nc.tensor.matmul(
                                            hps[:, :sn],
                                            w1_sb[e][:, 2 * k : 2 * k + 2,
                                                     m * 128 : (m + 1) * 128],
                                            x8_sb[:, 2 * k : 2 * k + 2,
                                                  off : off + sn],
                                            start=(k == 0),
                                            stop=(k == KC // 2 - 1
                                                  and not SILU_DECOMP),
                                            perf_mode=DR,
                                        )
                                    h_sb = hp_pool.tile(
                                        [128, TILE_N], BF16,
                                        name="h_sb", tag="h"
                                    )
                                    silu_to(
                                        h_sb[:, :sn], hps[:, :sn],
                                        e, m, 1.0 / WS,
                                    )
                                    ac = e * MC + m
                                    nc.vector.scalar_tensor_tensor(
                                        hpm8[:, j, :sn],
                                        h_sb[:, :sn],
                                        alpha_sb[:, ac : ac + 1],
                                        wsb[gi, e][:, :sn],
                                        mybir.AluOpType.add,
                                        mybir.AluOpType.mult,
                                    )
                                first = seg_start and ei == 0 and kk == 0
                                last = (stop_last and last_seg
                                        and ei == len(ee) - 1
                                        and kk == MC // 2 - 1)
                                for c in range(OC):
                                    nc.tensor.matmul(
                                        outps[c][:, off : off + sn],
                                        w2_sb[e][:, 2 * kk : 2 * kk + 2,
                                                 c * 128 : (c + 1) * 128],
                                        hpm8[:, :, :sn],
                                        start=first,
                                        stop=last,
                                        perf_mode=DR,
                                    )

                def emit_shared(start, stop_last):
                    # bf16 experts over the full tile (W2 carries the x16
                    # scale)
                    for si in range(N_SHARED):
                        e = N_ROUTED + si
                        for m in range(MC):
                            hps = hpsp.tile(
                                [128, TILE_N], F32, name="hps", tag="hps"
                            )
                            for k in range(KC):
                                nc.tensor.matmul(
                                    hps[:, :n],
                                    w1_sb[e][:, k, m * 128 : (m + 1) * 128],
                                    x_sb[:, k, :n],
                                    start=(k == 0),
                                    stop=(k == KC - 1 and not SILU_DECOMP),
                                )
                            h_sb = hp_pool.tile(
                                [128, TILE_N], BF16, name="h_sb", tag="h"
                            )
                            silu_to(h_sb[:, :n], hps[:, :n], e, m, 1.0)
                            first = start and si == 0 and m == 0
                            last = (stop_last and si == N_SHARED - 1
                                    and m == MC - 1)
                            for c in range(OC):
                                nc.tensor.matmul(
                                    outps[c][:, :n],
                                    w2_sb[e][:, m, c * 128 : (c + 1) * 128],
                                    h_sb[:, :n],
                                    start=first,
                                    stop=last,
                                )

                emit_shared(start=True, stop_last=False)
                emit_routed(seg_start=False, stop_last=True)

                # psum -> sbuf (+ shared-expert b2, x16) on DVE: keeps the out
                # path off the in-order ScalarE queue, which lags on silus.
                # Per-c copy + DMA so c=0 streams out while c=1 copies.
                osb = osb_pool.tile([128, OC, TILE_N], F32, name="osb", tag="osb")
                for c in range(OC):
                    nc.vector.tensor_scalar_add(
                        osb[:, c, :n], outps[c][:, :n], b2s_sb[:, c : c + 1]
                    )
                    nc.sync.dma_start(
                        outT_v[:, c, a0 : a0 + n], osb[:, c, :n]
                    )
                a0 += n

    nc.compile()
    return nc


def _alpha_pack(rW2, rb2):
    """alpha_e = min-norm solution of W2_e @ alpha = b2_e, packed per-chunk."""
    alphas = []
    for e in range(N_ROUTED):
        a, *_ = np.linalg.lstsq(rW2[e].astype(np.float64), rb2[e].astype(np.float64))
        alphas.append(a)
    al = np.stack(alphas).astype(np.float32)  # [6, HID]
    return np.ascontiguousarray(
        al.reshape(N_ROUTED, MC, 128).transpose(2, 0, 1).reshape(128, N_ROUTED * MC)
    )


def _prep_host(inputs):
    feats = np.asarray(inputs["features"], dtype=np.float32)
    species = np.asarray(inputs["species_idx"]).astype(np.int64)
    emb = np.asarray(inputs["emb"], dtype=np.float32)
    Wr = np.asarray(inputs["W_router"], dtype=np.float32)
    rW1 = np.asarray(inputs["rW1"], dtype=np.float32)
    rb1 = np.asarray(inputs["rb1"], dtype=np.float32)
    rW2 = np.asarray(inputs["rW2"], dtype=np.float32)
    rb2 = np.asarray(inputs["rb2"], dtype=np.float32)
    sW1 = np.asarray(inputs["sW1"], dtype=np.float32)
    sb1 = np.asarray(inputs["sb1"], dtype=np.float32)
    sW2 = np.asarray(inputs["sW2"], dtype=np.float32)
    sb2 = np.asarray(inputs["sb2"], dtype=np.float32)

    wt_table, top2 = _router_table(emb, Wr)
    idx_cores, tiles = _plan_sharding(species, top2)
    nl = idx_cores.shape[1]
    w_atoms = wt_table[species]  # [n, 6] f32

    b1 = np.concatenate([rb1, sb1], axis=0)  # [8, HID]
    # decomp-path row bias: routed entries carry the x16 PSUM scale
    b1r = b1.copy()
    b1r[:N_ROUTED] *= WS

    shared = {
        "w1t": np.ascontiguousarray(sW1.transpose(0, 2, 1)).astype(BF16_NP),
        "w2t": np.ascontiguousarray(sW2.transpose(0, 2, 1) * WS).astype(BF16_NP),
        "w1t8": np.ascontiguousarray(rW1.transpose(0, 2, 1) * WS).astype(FP8_NP),
        "w2t8": np.ascontiguousarray(rW2.transpose(0, 2, 1) * WS).astype(FP8_NP),
        "b1": np.ascontiguousarray(
            b1.reshape(N_EXP, MC, 128).transpose(2, 0, 1).reshape(128, N_EXP * MC)
        ),
        "b1r": b1r.reshape(1, N_EXP * HID).astype(BF16_NP),
        "alpha": _alpha_pack(rW2, rb2),
        "b2s": np.ascontiguousarray((sb2.sum(axis=0) * WS).reshape(OC, 128).T),
    }

    in_maps = []
    for c in range(N_CORES):
        idx = idx_cores[c]
        valid = idx >= 0
        iv = idx[valid]
        fT = np.ascontiguousarray(feats[iv].T)
        xT = np.zeros((IN_F, nl), dtype=BF16_NP)
        xT[:, valid] = fT.astype(BF16_NP)
        x8 = np.zeros((IN_F, nl), dtype=FP8_NP)
        x8[:, valid] = fT.astype(FP8_NP)
        w6 = np.zeros((N_ROUTED, nl), dtype=BF16_NP)
        w6[:, valid] = np.ascontiguousarray(w_atoms[iv].T).astype(BF16_NP)
        in_maps.append({"xT": xT, "x8": x8, "w6": w6, **shared})
    return in_maps, idx_cores, tiles, nl, feats.shape[0]


_PROGRAM_CACHE = {}


def _get_program(nl, tiles):
    key = (nl, tuple(tiles))
    if key not in _PROGRAM_CACHE:
        _PROGRAM_CACHE[key] = _build_program(nl, tiles)
    return _PROGRAM_CACHE[key]


# Set TRACE=True (e.g. from a test harness) to capture a neuron-profile trace;
# the full BassKernelResults of the last run is kept in LAST_RESULTS.
TRACE = False
LAST_RESULTS = None


def kernel(**inputs):
    global LAST_RESULTS
    in_maps, idx_cores, tiles, nl, n_atoms = _prep_host(inputs)
    nc = _get_program(nl, tiles)
    res = run_bass_kernel_spmd(nc, in_maps, list(range(N_CORES)), trace=TRACE)
    LAST_RESULTS = res
    out = np.zeros((n_atoms, OUT_F), dtype=np.float32)
    inv = np.float32(1.0 / WS)
    for c in range(N_CORES):
        idx = idx_cores[c]
        valid = idx >= 0
        outT = res.results[c]["outT"]  # [OUT_F, nl] f32, x16 scale
        out[idx[valid]] = outT[:, valid].T * inv
    return out


# revision 20
# speedup vs baseline: 1.0358x; 1.0358x over previous
"""Trainium2 Bass kernel for a species-routed MoE readout layer.

Math (see problem reference): per atom x [512]:
  u = silu(emb[species]); scores = softmax(u @ Wr.T)  -> top-2 sparse gates
  out = sum_e gate_e * (W2_e @ silu(W1_e @ x + b1_e) + b2_e)
      + sum_s (W2_s @ silu(W1_s @ x + b1_s) + b2_s)          # 2 shared experts

The router depends only on species_idx (64 species), so the per-atom top-2
gates collapse to a host-computed 64x6 lookup table. Atoms are grouped by
their top-2 expert pair and each group is split evenly across the 8 cores so
the single SPMD program sees the same tile->active-expert pattern on every
core; interior tiles then only compute 2 routed + 2 shared expert MLPs.

Precision split (validated on host, fp8_sim.py): the routed contributions
are gated by ~1/6-scale softmax scores while the shared experts carry full
weight, so the two ROUTED expert MLPs run in fp8-e4m3 with DoubleRow matmuls
(2 fp8 weights/PE cell -> 2x matmul FLOP rate, contraction 256/instruction)
while the two SHARED experts stay bf16. Routed weights are scaled x16 into
e4m3's normal range; the x16 on W1 is undone by the ScalarE Silu's scale
operand, the x16 on W2 rides the PSUM and is matched by scaling the shared
W2 (bf16, exact) and b2 by 16 too; the host divides the output by 16.
Measured max-rel error ~9e-3 vs the 2e-2 gate (bf16 baseline: 3.4e-3).

Device side (per core, per <=512-atom variable-size tile, tiles aligned to
expert-pair segment boundaries):
  routed e: hps = sum_k DoubleRowMM(16*W1T8[e], x8) ;
            h = Silu(hps/16 + b1) (ScalarE) ;
            hpm8[j] = (h + alpha_e) * gate  (DVE STT, fp8 out; alpha_e =
            lstsq(W2_e, b2_e) folds the gated b2 into the gate multiply) ;
            outps += DoubleRowMM(16*W2T8[e], hpm8 pair)
  shared s: bf16 matmuls as before, W2 pre-scaled x16.
Shared-expert b2 sum (x16) is added on the PSUM->SBUF copy (DVE).
A short PE spin + dummy activation at kernel start warms the HAM clock
gate and the ACT table while the first DMAs are in flight; per-expert
weight tiles stream in first-use order behind tile 0's inputs.
"""

import numpy as np
import ml_dtypes

import concourse.bass as bass
import concourse.mybir as mybir
from concourse import bacc, tile
from concourse.bass_utils import run_bass_kernel_spmd

BF16 = mybir.dt.bfloat16
FP8 = mybir.dt.float8e4
F32 = mybir.dt.float32
BF16_NP = ml_dtypes.bfloat16
FP8_NP = ml_dtypes.float8_e4m3

N_CORES = 8
N_ATOMS = 100000
IN_F = 512
HID = 512
OUT_F = 256
N_ROUTED = 6
N_SHARED = 2
N_EXP = N_ROUTED + N_SHARED
TOPK = 2
TILE_N = 512  # atoms per tile = one PSUM bank = max matmul moving dim
KC = IN_F // 128   # 4 contraction chunks for mm1
MC = HID // 128    # 4 hid chunks
OC = OUT_F // 128  # 2 out chunks
WS = 16.0  # fp8 routed weight scale (power of 2; undone via Silu scale/host)

SPARSE = True  # compute only active routed experts per tile
# CoreSim has no Silu: decompose as x*sigmoid(x) (bias folded into PSUM via a
# rank-1 matmul). HW path uses native ScalarE Silu with the f32 bias operand.
SILU_DECOMP = False
DR = mybir.MatmulPerfMode.DoubleRow


def _silu(x):
    return x / (1.0 + np.exp(-x))


def _router_table(emb, W_router):
    """[64, 6] sparse top-2 gate table + per-species expert pair."""
    u = _silu(emb.astype(np.float32))
    logits = u @ W_router.astype(np.float32).T
    m = logits.max(axis=-1, keepdims=True)
    e = np.exp(logits - m)
    scores = e / e.sum(axis=-1, keepdims=True)
    order = np.argsort(-scores, axis=-1, kind="stable")
    top2 = order[:, :TOPK]
    wt = np.zeros_like(scores)
    rows = np.arange(scores.shape[0])[:, None]
    wt[rows, top2] = scores[rows, top2]
    return wt, top2


def _plan_sharding(species_idx, top2):
    """Group atoms by top-2 expert pair, split each group evenly over cores.

    Returns (idx_cores [N_CORES, NL] int64 with -1 padding, tiles) where
    tiles is a list of (n_atoms, segments) per full-width tile and segments
    is a tuple of (col_offset, n_cols, routed_experts); identical for every
    core by construction. Tiles are exactly TILE_N wide (except the last):
    the shared experts run on the full tile while the routed experts run on
    per-segment column ranges, so merging expert-pair segments into full
    tiles costs no extra routed compute.
    """
    n = species_idx.shape[0]
    if not SPARSE:
        assert n % N_CORES == 0
        nl = n // N_CORES
        idx_cores = np.arange(n, dtype=np.int64).reshape(N_CORES, nl)
        tiles = []
        for t0 in range(0, nl, TILE_N):
            w = min(TILE_N, nl - t0)
            tiles.append((w, ((0, w, tuple(range(N_ROUTED))),)))
        return idx_cores, tiles

    MIN_TILE = 64  # merge segments smaller than this into their neighbor

    pair_of_species = [tuple(sorted(top2[s])) for s in range(top2.shape[0])]
    pairs = sorted(set(pair_of_species))
    pair_id_of_species = np.array(
        [pairs.index(p) for p in pair_of_species], dtype=np.int64
    )
    atom_pair = pair_id_of_species[species_idx]

    seg_lens = []       # per-group per-core segment length
    seg_experts = []
    group_idx = []      # per-group atom index arrays
    for g, p in enumerate(pairs):
        idx_g = np.nonzero(atom_pair == g)[0]
        if idx_g.size == 0:
            continue
        L = -(-idx_g.size // N_CORES)  # ceil
        seg_lens.append(L)
        seg_experts.append(tuple(int(x) for x in p))
        group_idx.append(idx_g)

    # largest group first: deep pipeline while the clock warms, short tail
    order = np.argsort([-L for L in seg_lens], kind="stable")
    seg_lens = [seg_lens[i] for i in order]
    seg_experts = [seg_experts[i] for i in order]
    group_idx = [group_idx[i] for i in order]

    nl = sum(seg_lens)
    idx_cores = np.full((N_CORES, nl), -1, dtype=np.int64)
    off = 0
    for L, idx_g in zip(seg_lens, group_idx):
        for c in range(N_CORES):
            part = idx_g[c * L : (c + 1) * L]
            idx_cores[c, off : off + part.size] = part
        off += L

    # Split each expert-pair group into near-equal pieces <= TILE_N (tiny
    # groups union-merge into their neighbor). Splitting a pair mid-group
    # into tiny segments is a loss (DoubleRow matmuls have a ~135ns
    # LDWEIGHTS floor regardless of width), so pieces stay whole; adjacent
    # whole pieces that fit in one tile merge as separate segments, sharing
    # the tile's shared-expert matmuls.
    pieces = []  # (length, experts) in column-layout order
    pend_n, pend_e = 0, set()
    for L, p in zip(seg_lens, seg_experts):
        pend_n += L
        pend_e.update(p)
        if pend_n < MIN_TILE:
            continue
        k = -(-pend_n // TILE_N)
        q, r = divmod(pend_n, k)
        for i in range(k):
            pieces.append((q + (1 if i < r else 0), tuple(sorted(pend_e))))
        pend_n, pend_e = 0, set()
    if pend_n:
        if pieces:
            n0, e0 = pieces.pop()
            pend_n += n0
            pend_e.update(e0)
        k = -(-pend_n // TILE_N)
        q, r = divmod(pend_n, k)
        ee = tuple(sorted(pend_e))
        for i in range(k):
            pieces.append((q + (1 if i < r else 0), ee))

    tiles = []
    for pi, (L, ee) in enumerate(pieces):
        # never merge into tile 0: it runs routed-first (smallest startup
        # DMAs), which needs a single full-width PSUM start group
        if len(tiles) > 1 and tiles[-1][0] + L <= TILE_N:
            n0, segs0 = tiles[-1]
            tiles[-1] = (n0 + L, segs0 + ((n0, L, ee),))
        else:
            tiles.append((L, ((0, L, ee),)))
    assert sum(t[0] for t in tiles) == nl
    return idx_cores, tiles


def _build_program(nl, tiles):
    nc = bacc.Bacc("TRN2", target_bir_lowering=False, debug=False)

    xT_d = nc.declare_dram_parameter("xT", [IN_F, nl], BF16, isOutput=False)
    x8_d = nc.declare_dram_parameter("x8", [IN_F, nl], FP8, isOutput=False)
    w6_d = nc.declare_dram_parameter("w6", [N_ROUTED, nl], BF16, isOutput=False)
    # shared expert weights, bf16 (W2 pre-scaled x16 on host)
    w1t_d = nc.declare_dram_parameter(
        "w1t", [N_SHARED, IN_F, HID], BF16, isOutput=False
    )
    w2t_d = nc.declare_dram_parameter(
        "w2t", [N_SHARED, HID, OUT_F], BF16, isOutput=False
    )
    # routed expert weights, fp8 e4m3, scaled x16
    w1t8_d = nc.declare_dram_parameter(
        "w1t8", [N_ROUTED, IN_F, HID], FP8, isOutput=False
    )
    w2t8_d = nc.declare_dram_parameter(
        "w2t8", [N_ROUTED, HID, OUT_F], FP8, isOutput=False
    )
    b1_d = nc.declare_dram_parameter("b1", [128, N_EXP * MC], F32, isOutput=False)
    b1r_d = nc.declare_dram_parameter("b1r", [1, N_EXP * HID], BF16, isOutput=False)
    # alpha[e] solves W2_e @ alpha_e = b2_e (host lstsq), so the gated b2
    # rides the gate multiply: W2_e @ (w*(h+alpha)) = w*(W2_e h) + w*b2_e
    alpha_d = nc.declare_dram_parameter(
        "alpha", [128, N_ROUTED * MC], F32, isOutput=False
    )
    b2s_d = nc.declare_dram_parameter("b2s", [128, OC], F32, isOutput=False)
    outT_d = nc.declare_dram_parameter("outT", [OUT_F, nl], F32, isOutput=True)

    with tile.TileContext(nc) as tc:
        with (
            tc.tile_pool(name="consts", bufs=1) as consts,
            tc.tile_pool(name="xp", bufs=3) as xp,
            tc.tile_pool(name="x8p", bufs=3) as x8p,
            tc.tile_pool(name="w6p", bufs=2) as w6p,
            tc.tile_pool(name="wbcp", bufs=10) as wbcp,
            tc.tile_pool(name="hps", bufs=4, space="PSUM") as hpsp,
            tc.tile_pool(name="hp", bufs=8) as hp_pool,
            tc.tile_pool(name="hpm8", bufs=4) as hpm8_pool,
            tc.tile_pool(name="ops", bufs=4, space="PSUM") as outps_pool,
            tc.tile_pool(name="osb", bufs=4) as osb_pool,
        ):
            # ---- constants / weights preload ----
            b1_sb = consts.tile([128, N_EXP * MC], F32, name="b1_sb")
            alpha_sb = consts.tile([128, N_ROUTED * MC], F32, name="alpha_sb")
            b2s_sb = consts.tile([128, OC], F32, name="b2s_sb")
            ones_sb = consts.tile([1, 128], BF16, name="ones_sb")
            b1row_sb = consts.tile([1, N_EXP * HID], BF16, name="b1row_sb")
            ones_row = consts.tile([1, TILE_N], BF16, name="ones_row")

            nc.sync.dma_start(b1_sb[:], b1_d[:])
            nc.sync.dma_start(alpha_sb[:], alpha_d[:])
            nc.sync.dma_start(b2s_sb[:], b2s_d[:])
            nc.sync.dma_start(b1row_sb[:], b1r_d[:])
            nc.vector.memset(ones_sb[:], 1.0)
            nc.vector.memset(ones_row[:], 1.0)

            # Per-expert weight tiles, loaded in first-use order. Tile-granular
            # deps let tile 0's matmuls start after the first expert arrives.
            # tile 0 runs routed-first (fp8 weights + x8 are the smallest
            # startup DMAs), later tiles shared-first; weights load in
            # first-use order
            eorder = []
            for _, segments in tiles:
                for _, _, ee in segments:
                    for e in list(ee) + [N_ROUTED + s for s in range(N_SHARED)]:
                        if e not in eorder:
                            eorder.append(e)
                if len(eorder) == N_EXP:
                    break
            for e in range(N_EXP):
                if e not in eorder:
                    eorder.append(e)

            # one big strided DMA per expert per matrix (queue-issue cost is
            # per-op, transfers run on the parallel DMA engines)
            w1t_v = w1t_d.rearrange("e (k p) h -> e p k h", p=128)
            w2t_v = w2t_d.rearrange("e (m p) o -> e p m o", p=128)
            w1t8_v = w1t8_d.rearrange("e (k p) h -> e p k h", p=128)
            w2t8_v = w2t8_d.rearrange("e (m p) o -> e p m o", p=128)
            w1_sb = {}
            w2_sb = {}

            def load_expert_weights(e):
                if e < N_ROUTED:
                    w1_sb[e] = consts.tile([128, KC, HID], FP8, name=f"w1e{e}")
                    w2_sb[e] = consts.tile([128, MC, OUT_F], FP8, name=f"w2e{e}")
                    nc.sync.dma_start(w1_sb[e][:], w1t8_v[e])
                    nc.sync.dma_start(w2_sb[e][:], w2t8_v[e])
                else:
                    s = e - N_ROUTED
                    w1_sb[e] = consts.tile([128, KC, HID], BF16, name=f"w1e{e}")
                    w2_sb[e] = consts.tile([128, MC, OUT_F], BF16, name=f"w2e{e}")
                    nc.sync.dma_start(w1_sb[e][:], w1t_v[s])
                    nc.sync.dma_start(w2_sb[e][:], w2t_v[s])

            load_expert_weights(eorder[0])

            # Warm the PE HAM clock gate (cold = 1.2 GHz until ~3.4us of
            # sustained activity) and the ScalarE activation table while the
            # first input DMAs are in flight.
            warm_sb = consts.tile([128, 1], F32, name="warm_sb")
            for _ in range(12):
                warm_ps = hpsp.tile([128, TILE_N], F32, name="warm_ps", tag="hps")
                nc.tensor.matmul(
                    warm_ps[:, :], ones_sb[:, :], ones_row[0:1, :],
                    start=True, stop=True,
                )
            nc.scalar.activation(
                warm_sb[:, :], b1_sb[:, 0:1],
                mybir.ActivationFunctionType.Sigmoid
                if SILU_DECOMP
                else mybir.ActivationFunctionType.Silu,
            )

            # ---- main loop over atom tiles ----
            xT_v = xT_d.rearrange("(k p) a -> p k a", p=128)
            x8_v = x8_d.rearrange("(k p) a -> p k a", p=128)
            outT_v = outT_d.rearrange("(c p) a -> p c a", p=128)
            a0 = 0
            for t, (n, segments) in enumerate(tiles):
                # routed-first needs one full-width PSUM start group, so it
                # requires a single-segment tile; only used for tile 0
                routed_first = t == 0 and len(segments) == 1
                x_sb = xp.tile([128, KC, TILE_N], BF16, name="x_sb", tag="x")
                x8_sb = x8p.tile([128, KC, TILE_N], FP8, name="x8_sb", tag="x8")
                w6row = w6p.tile([1, N_ROUTED, TILE_N], BF16, name="w6row", tag="w6r")
                if routed_first:
                    nc.sync.dma_start(x8_sb[:, :, :n], x8_v[:, :, a0 : a0 + n])
                    nc.sync.dma_start(w6row[0:1, :, :n], w6_d[:, a0 : a0 + n])
                    nc.sync.dma_start(x_sb[:, :, :n], xT_v[:, :, a0 : a0 + n])
                else:
                    nc.sync.dma_start(x_sb[:, :, :n], xT_v[:, :, a0 : a0 + n])
                    nc.sync.dma_start(x8_sb[:, :, :n], x8_v[:, :, a0 : a0 + n])
                    nc.sync.dma_start(w6row[0:1, :, :n], w6_d[:, a0 : a0 + n])

                if t == 0:
                    # stream the remaining experts' weights behind tile 0's
                    # inputs; tile 0's compute covers the transfer time
                    for e in eorder[1:]:
                        load_expert_weights(e)

                # per-atom gates broadcast across 128 partitions (GPSIMD,
                # keeps PE free); one broadcast per (segment, expert)
                wsb = {}
                for gi, (off, sn, ee) in enumerate(segments):
                    for e in ee:
                        wsb_e = wbcp.tile(
                            [128, TILE_N], BF16, name="wsb", tag="wbc"
                        )
                        nc.gpsimd.partition_broadcast(
                            wsb_e[:, :sn], w6row[0:1, e, off : off + sn]
                        )
                        wsb[gi, e] = wsb_e

                # output accumulators (PSUM values carry a x16 scale)
                outps = [
                    outps_pool.tile([128, TILE_N], F32, name="ops", tag="ops")
                    for _ in range(OC)
                ]

                def silu_to(h_ap, hps_ap, e, m, scale):
                    w = h_ap.shape[-1]
                    if SILU_DECOMP:
                        boff = e * HID + m * 128
                        nc.tensor.matmul(
                            hps_ap,
                            b1row_sb[0:1, boff : boff + 128],
                            ones_row[0:1, :w],
                            start=False, stop=True,
                        )
                        s_sb = hp_pool.tile(
                            [128, TILE_N], BF16, name="s_sb", tag="s"
                        )
                        nc.scalar.activation(
                            s_sb[:, :w], hps_ap,
                            mybir.ActivationFunctionType.Sigmoid,
                            scale=scale,
                        )
                        nc.vector.scalar_tensor_tensor(
                            h_ap, hps_ap, scale, s_sb[:, :w],
                            mybir.AluOpType.mult, mybir.AluOpType.mult,
                        )
                    else:
                        nc.scalar.activation(
                            h_ap, hps_ap,
                            mybir.ActivationFunctionType.Silu,
                            bias=b1_sb[:, e * MC + m : e * MC + m + 1],
                            scale=scale,
                        )

                def emit_routed(seg_start, stop_last):
                    # fp8 e4m3 DoubleRow experts, per column segment. With
                    # seg_start, the first mm2 group of each segment clears
                    # its PSUM subrange (per-element has_written).
                    for gi, (off, sn, ee) in enumerate(segments):
                        last_seg = gi == len(segments) - 1
                        for ei, e in enumerate(ee):
                            for kk in range(MC // 2):  # m-chunk pairs
                                hpm8 = hpm8_pool.tile(
                                    [128, 2, TILE_N], FP8,
                                    name="hpm8", tag="hpm8"
                                )
                                for j in range(2):
                                    m = 2 * kk + j
                                    hps = hpsp.tile(
                                        [128, TILE_N], F32,
                                        name="hps", tag="hps"
                                    )
                                    for k in range(KC // 2):
                                        nc.tensor.matmul(
                                            hps[:, :sn],
                                            w1_sb[e][:, 2 * k : 2 * k + 2,
                                                     m * 128 : (m + 1) * 128],
                                            x8_sb[:, 2 * k : 2 * k + 2,
                                                  off : off + sn],
                                            start=(k == 0),
                                            stop=(k == KC // 2 - 1
                                                  and not SILU_DECOMP),
                                            perf_mode=DR,
                                        )
                                    h_sb = hp_pool.tile(
                                        [128, TILE_N], BF16,
                                        name="h_sb", tag="h"
                                    )
                                    silu_to(
                                        h_sb[:, :sn], hps[:, :sn],
                                        e, m, 1.0 / WS,
                                    )
                                    ac = e * MC + m
                                    nc.vector.scalar_tensor_tensor(
                                        hpm8[:, j, :sn],
                                        h_sb[:, :sn],
                                        alpha_sb[:, ac : ac + 1],
                                        wsb[gi, e][:, :sn],
                                        mybir.AluOpType.add,
                                        mybir.AluOpType.mult,
                                    )
                                first = seg_start and ei == 0 and kk == 0
                                last = (stop_last and last_seg
                                        and ei == len(ee) - 1
                                        and kk == MC // 2 - 1)
                                for c in range(OC):
                                    nc.tensor.matmul(
                                        outps[c][:, off : off + sn],
                                        w2_sb[e][:, 2 * kk : 2 * kk + 2,
                                                 c * 128 : (c + 1) * 128],
                                        hpm8[:, :, :sn],
                                        start=first,
                                        stop=last,
                                        perf_mode=DR,
                                    )

                def emit_shared(start, stop_last):
                    # bf16 experts over the full tile (W2 carries the x16
                    # scale)
                    for si in range(N_SHARED):
                        e = N_ROUTED + si
                        for m in range(MC):
                            hps = hpsp.tile(
                                [128, TILE_N], F32, name="hps", tag="hps"
                            )
                            for k in range(KC):
                                nc.tensor.matmul(
                                    hps[:, :n],
                                    w1_sb[e][:, k, m * 128 : (m + 1) * 128],
                                    x_sb[:, k, :n],
                                    start=(k == 0),
                                    stop=(k == KC - 1 and not SILU_DECOMP),
                                )
                            h_sb = hp_pool.tile(
                                [128, TILE_N], BF16, name="h_sb", tag="h"
                            )
                            silu_to(h_sb[:, :n], hps[:, :n], e, m, 1.0)
                            first = start and si == 0 and m == 0
                            last = (stop_last and si == N_SHARED - 1
                                    and m == MC - 1)
                            for c in range(OC):
                                nc.tensor.matmul(
                                    outps[c][:, :n],
                                    w2_sb[e][:, m, c * 128 : (c + 1) * 128],
                                    h_sb[:, :n],
                                    start=first,
                                    stop=last,
                                )

                if routed_first:
                    emit_routed(seg_start=True, stop_last=False)
                    emit_shared(start=False, stop_last=True)
                else:
                    emit_shared(start=True, stop_last=False)
                    emit_routed(seg_start=False, stop_last=True)

                # psum -> sbuf (+ shared-expert b2, x16) on DVE: keeps the out
                # path off the in-order ScalarE queue, which lags on silus.
                # Per-c copy + DMA so c=0 streams out while c=1 copies.
                osb = osb_pool.tile([128, OC, TILE_N], F32, name="osb", tag="osb")
                for c in range(OC):
                    nc.vector.tensor_scalar_add(
                        osb[:, c, :n], outps[c][:, :n], b2s_sb[:, c : c + 1]
                    )
                    nc.sync.dma_start(
                        outT_v[:, c, a0 : a0 + n], osb[:, c, :n]
                    )
                a0 += n

    nc.compile()
    return nc


def _alpha_pack(rW2, rb2):
    """alpha_e = min-norm solution of W2_e @ alpha = b2_e, packed per-chunk."""
    alphas = []
    for e in range(N_ROUTED):
        a, *_ = np.linalg.lstsq(rW2[e].astype(np.float64), rb2[e].astype(np.float64))
        alphas.append(a)
    al = np.stack(alphas).astype(np.float32)  # [6, HID]
    return np.ascontiguousarray(
        al.reshape(N_ROUTED, MC, 128).transpose(2, 0, 1).reshape(128, N_ROUTED * MC)
    )


def _prep_host(inputs):
    feats = np.asarray(inputs["features"], dtype=np.float32)
    species = np.asarray(inputs["species_idx"]).astype(np.int64)
    emb = np.asarray(inputs["emb"], dtype=np.float32)
    Wr = np.asarray(inputs["W_router"], dtype=np.float32)
    rW1 = np.asarray(inputs["rW1"], dtype=np.float32)
    rb1 = np.asarray(inputs["rb1"], dtype=np.float32)
    rW2 = np.asarray(inputs["rW2"], dtype=np.float32)
    rb2 = np.asarray(inputs["rb2"], dtype=np.float32)
    sW1 = np.asarray(inputs["sW1"], dtype=np.float32)
    sb1 = np.asarray(inputs["sb1"], dtype=np.float32)
    sW2 = np.asarray(inputs["sW2"], dtype=np.float32)
    sb2 = np.asarray(inputs["sb2"], dtype=np.float32)

    wt_table, top2 = _router_table(emb, Wr)
    idx_cores, tiles = _plan_sharding(species, top2)
    nl = idx_cores.shape[1]
    w_atoms = wt_table[species]  # [n, 6] f32

    b1 = np.concatenate([rb1, sb1], axis=0)  # [8, HID]
    # decomp-path row bias: routed entries carry the x16 PSUM scale
    b1r = b1.copy()
    b1r[:N_ROUTED] *= WS

    shared = {
        "w1t": np.ascontiguousarray(sW1.transpose(0, 2, 1)).astype(BF16_NP),
        "w2t": np.ascontiguousarray(sW2.transpose(0, 2, 1) * WS).astype(BF16_NP),
        "w1t8": np.ascontiguousarray(rW1.transpose(0, 2, 1) * WS).astype(FP8_NP),
        "w2t8": np.ascontiguousarray(rW2.transpose(0, 2, 1) * WS).astype(FP8_NP),
        "b1": np.ascontiguousarray(
            b1.reshape(N_EXP, MC, 128).transpose(2, 0, 1).reshape(128, N_EXP * MC)
        ),
        "b1r": b1r.reshape(1, N_EXP * HID).astype(BF16_NP),
        "alpha": _alpha_pack(rW2, rb2),
        "b2s": np.ascontiguousarray((sb2.sum(axis=0) * WS).reshape(OC, 128).T),
    }

    in_maps = []
    for c in range(N_CORES):
        idx = idx_cores[c]
        valid = idx >= 0
        iv = idx[valid]
        fT = np.ascontiguousarray(feats[iv].T)
        xT = np.zeros((IN_F, nl), dtype=BF16_NP)
        xT[:, valid] = fT.astype(BF16_NP)
        x8 = np.zeros((IN_F, nl), dtype=FP8_NP)
        x8[:, valid] = fT.astype(FP8_NP)
        w6 = np.zeros((N_ROUTED, nl), dtype=BF16_NP)
        w6[:, valid] = np.ascontiguousarray(w_atoms[iv].T).astype(BF16_NP)
        in_maps.append({"xT": xT, "x8": x8, "w6": w6, **shared})
    return in_maps, idx_cores, tiles, nl, feats.shape[0]


_PROGRAM_CACHE = {}


def _get_program(nl, tiles):
    key = (nl, tuple(tiles))
    if key not in _PROGRAM_CACHE:
        _PROGRAM_CACHE[key] = _build_program(nl, tiles)
    return _PROGRAM_CACHE[key]


# Set TRACE=True (e.g. from a test harness) to capture a neuron-profile trace;
# the full BassKernelResults of the last run is kept in LAST_RESULTS.
TRACE = False
LAST_RESULTS = None


def kernel(**inputs):
    global LAST_RESULTS
    in_maps, idx_cores, tiles, nl, n_atoms = _prep_host(inputs)
    nc = _get_program(nl, tiles)
    res = run_bass_kernel_spmd(nc, in_maps, list(range(N_CORES)), trace=TRACE)
    LAST_RESULTS = res
    out = np.zeros((n_atoms, OUT_F), dtype=np.float32)
    inv = np.float32(1.0 / WS)
    for c in range(N_CORES):
        idx = idx_cores[c]
        valid = idx >= 0
        outT = res.results[c]["outT"]  # [OUT_F, nl] f32, x16 scale
        out[idx[valid]] = outT[:, valid].T * inv
    return out


# revision 22
# speedup vs baseline: 1.0362x; 1.0003x over previous
"""Trainium2 Bass kernel for a species-routed MoE readout layer.

Math (see problem reference): per atom x [512]:
  u = silu(emb[species]); scores = softmax(u @ Wr.T)  -> top-2 sparse gates
  out = sum_e gate_e * (W2_e @ silu(W1_e @ x + b1_e) + b2_e)
      + sum_s (W2_s @ silu(W1_s @ x + b1_s) + b2_s)          # 2 shared experts

The router depends only on species_idx (64 species), so the per-atom top-2
gates collapse to a host-computed 64x6 lookup table. Atoms are grouped by
their top-2 expert pair and each group is split evenly across the 8 cores so
the single SPMD program sees the same tile->active-expert pattern on every
core; interior tiles then only compute 2 routed + 2 shared expert MLPs.

Precision split (validated on host, fp8_sim.py): the routed contributions
are gated by ~1/6-scale softmax scores while the shared experts carry full
weight, so the two ROUTED expert MLPs run in fp8-e4m3 with DoubleRow matmuls
(2 fp8 weights/PE cell -> 2x matmul FLOP rate, contraction 256/instruction)
while the two SHARED experts stay bf16. Routed weights are scaled x16 into
e4m3's normal range; the x16 on W1 is undone by the ScalarE Silu's scale
operand, the x16 on W2 rides the PSUM and is matched by scaling the shared
W2 (bf16, exact) and b2 by 16 too; the host divides the output by 16.
Measured max-rel error ~9e-3 vs the 2e-2 gate (bf16 baseline: 3.4e-3).

Device side (per core, per <=512-atom variable-size tile, tiles aligned to
expert-pair segment boundaries):
  routed e: hps = sum_k DoubleRowMM(16*W1T8[e], x8) ;
            h = Silu(hps/16 + b1) (ScalarE) ;
            hpm8[j] = (h + alpha_e) * gate  (DVE STT, fp8 out; alpha_e =
            lstsq(W2_e, b2_e) folds the gated b2 into the gate multiply) ;
            outps += DoubleRowMM(16*W2T8[e], hpm8 pair)
  shared s: bf16 matmuls as before, W2 pre-scaled x16.
Shared-expert b2 sum (x16) is added on the PSUM->SBUF copy (DVE).
A short PE spin + dummy activation at kernel start warms the HAM clock
gate and the ACT table while the first DMAs are in flight; per-expert
weight tiles stream in first-use order behind tile 0's inputs.
"""

import numpy as np
import ml_dtypes

import concourse.bass as bass
import concourse.mybir as mybir
from concourse import bacc, tile
from concourse.bass_utils import run_bass_kernel_spmd

BF16 = mybir.dt.bfloat16
FP8 = mybir.dt.float8e4
F32 = mybir.dt.float32
BF16_NP = ml_dtypes.bfloat16
FP8_NP = ml_dtypes.float8_e4m3

N_CORES = 8
N_ATOMS = 100000
IN_F = 512
HID = 512
OUT_F = 256
N_ROUTED = 6
N_SHARED = 2
N_EXP = N_ROUTED + N_SHARED
TOPK = 2
TILE_N = 512  # atoms per tile = one PSUM bank = max matmul moving dim
KC = IN_F // 128   # 4 contraction chunks for mm1
MC = HID // 128    # 4 hid chunks
OC = OUT_F // 128  # 2 out chunks
WS = 16.0  # fp8 routed weight scale (power of 2; undone via Silu scale/host)

SPARSE = True  # compute only active routed experts per tile
# CoreSim has no Silu: decompose as x*sigmoid(x) (bias folded into PSUM via a
# rank-1 matmul). HW path uses native ScalarE Silu with the f32 bias operand.
SILU_DECOMP = False
DR = mybir.MatmulPerfMode.DoubleRow


def _silu(x):
    return x / (1.0 + np.exp(-x))


def _router_table(emb, W_router):
    """[64, 6] sparse top-2 gate table + per-species expert pair."""
    u = _silu(emb.astype(np.float32))
    logits = u @ W_router.astype(np.float32).T
    m = logits.max(axis=-1, keepdims=True)
    e = np.exp(logits - m)
    scores = e / e.sum(axis=-1, keepdims=True)
    order = np.argsort(-scores, axis=-1, kind="stable")
    top2 = order[:, :TOPK]
    wt = np.zeros_like(scores)
    rows = np.arange(scores.shape[0])[:, None]
    wt[rows, top2] = scores[rows, top2]
    return wt, top2


def _plan_sharding(species_idx, top2):
    """Group atoms by top-2 expert pair, split each group evenly over cores.

    Returns (idx_cores [N_CORES, NL] int64 with -1 padding, tiles) where
    tiles is a list of (n_atoms, segments) per full-width tile and segments
    is a tuple of (col_offset, n_cols, routed_experts); identical for every
    core by construction. Tiles are exactly TILE_N wide (except the last):
    the shared experts run on the full tile while the routed experts run on
    per-segment column ranges, so merging expert-pair segments into full
    tiles costs no extra routed compute.
    """
    n = species_idx.shape[0]
    if not SPARSE:
        assert n % N_CORES == 0
        nl = n // N_CORES
        idx_cores = np.arange(n, dtype=np.int64).reshape(N_CORES, nl)
        tiles = []
        for t0 in range(0, nl, TILE_N):
            w = min(TILE_N, nl - t0)
            tiles.append((w, ((0, w, tuple(range(N_ROUTED))),)))
        return idx_cores, tiles

    MIN_TILE = 64  # merge segments smaller than this into their neighbor

    pair_of_species = [tuple(sorted(top2[s])) for s in range(top2.shape[0])]
    pairs = sorted(set(pair_of_species))
    pair_id_of_species = np.array(
        [pairs.index(p) for p in pair_of_species], dtype=np.int64
    )
    atom_pair = pair_id_of_species[species_idx]

    seg_lens = []       # per-group per-core segment length
    seg_experts = []
    group_idx = []      # per-group atom index arrays
    for g, p in enumerate(pairs):
        idx_g = np.nonzero(atom_pair == g)[0]
        if idx_g.size == 0:
            continue
        L = -(-idx_g.size // N_CORES)  # ceil
        seg_lens.append(L)
        seg_experts.append(tuple(int(x) for x in p))
        group_idx.append(idx_g)

    # largest group first: deep pipeline while the clock warms, short tail
    order = np.argsort([-L for L in seg_lens], kind="stable")
    seg_lens = [seg_lens[i] for i in order]
    seg_experts = [seg_experts[i] for i in order]
    group_idx = [group_idx[i] for i in order]

    nl = sum(seg_lens)
    idx_cores = np.full((N_CORES, nl), -1, dtype=np.int64)
    off = 0
    for L, idx_g in zip(seg_lens, group_idx):
        for c in range(N_CORES):
            part = idx_g[c * L : (c + 1) * L]
            idx_cores[c, off : off + part.size] = part
        off += L

    # Split each expert-pair group into near-equal pieces <= TILE_N (tiny
    # groups union-merge into their neighbor). Splitting a pair mid-group
    # into tiny segments is a loss (DoubleRow matmuls have a ~135ns
    # LDWEIGHTS floor regardless of width), so pieces stay whole; adjacent
    # whole pieces that fit in one tile merge as separate segments, sharing
    # the tile's shared-expert matmuls.
    pieces = []  # (length, experts) in column-layout order
    pend_n, pend_e = 0, set()
    for L, p in zip(seg_lens, seg_experts):
        pend_n += L
        pend_e.update(p)
        if pend_n < MIN_TILE:
            continue
        k = -(-pend_n // TILE_N)
        q, r = divmod(pend_n, k)
        for i in range(k):
            pieces.append((q + (1 if i < r else 0), tuple(sorted(pend_e))))
        pend_n, pend_e = 0, set()
    if pend_n:
        if pieces:
            n0, e0 = pieces.pop()
            pend_n += n0
            pend_e.update(e0)
        k = -(-pend_n // TILE_N)
        q, r = divmod(pend_n, k)
        ee = tuple(sorted(pend_e))
        for i in range(k):
            pieces.append((q + (1 if i < r else 0), ee))

    tiles = []
    for pi, (L, ee) in enumerate(pieces):
        # never merge into tile 0: it runs routed-first (smallest startup
        # DMAs), which needs a single full-width PSUM start group
        if len(tiles) > 1 and tiles[-1][0] + L <= TILE_N:
            n0, segs0 = tiles[-1]
            tiles[-1] = (n0 + L, segs0 + ((n0, L, ee),))
        else:
            tiles.append((L, ((0, L, ee),)))
    assert sum(t[0] for t in tiles) == nl
    return idx_cores, tiles


def _build_program(nl, tiles):
    nc = bacc.Bacc("TRN2", target_bir_lowering=False, debug=False)

    xT_d = nc.declare_dram_parameter("xT", [IN_F, nl], BF16, isOutput=False)
    x8_d = nc.declare_dram_parameter("x8", [IN_F, nl], FP8, isOutput=False)
    w6_d = nc.declare_dram_parameter("w6", [N_ROUTED, nl], BF16, isOutput=False)
    # shared expert weights, bf16 (W2 pre-scaled x16 on host)
    w1t_d = nc.declare_dram_parameter(
        "w1t", [N_SHARED, IN_F, HID], BF16, isOutput=False
    )
    w2t_d = nc.declare_dram_parameter(
        "w2t", [N_SHARED, HID, OUT_F], BF16, isOutput=False
    )
    # routed expert weights, fp8 e4m3, scaled x16
    w1t8_d = nc.declare_dram_parameter(
        "w1t8", [N_ROUTED, IN_F, HID], FP8, isOutput=False
    )
    w2t8_d = nc.declare_dram_parameter(
        "w2t8", [N_ROUTED, HID, OUT_F], FP8, isOutput=False
    )
    b1_d = nc.declare_dram_parameter("b1", [128, N_EXP * MC], F32, isOutput=False)
    b1r_d = nc.declare_dram_parameter("b1r", [1, N_EXP * HID], BF16, isOutput=False)
    # alpha[e] solves W2_e @ alpha_e = b2_e (host lstsq), so the gated b2
    # rides the gate multiply: W2_e @ (w*(h+alpha)) = w*(W2_e h) + w*b2_e
    alpha_d = nc.declare_dram_parameter(
        "alpha", [128, N_ROUTED * MC], F32, isOutput=False
    )
    b2s_d = nc.declare_dram_parameter("b2s", [128, OC], F32, isOutput=False)
    outT_d = nc.declare_dram_parameter("outT", [OUT_F, nl], F32, isOutput=True)

    with tile.TileContext(nc) as tc:
        with (
            tc.tile_pool(name="consts", bufs=1) as consts,
            tc.tile_pool(name="xp", bufs=3) as xp,
            tc.tile_pool(name="x8p", bufs=3) as x8p,
            tc.tile_pool(name="w6p", bufs=2) as w6p,
            tc.tile_pool(name="wbcp", bufs=10) as wbcp,
            tc.tile_pool(name="hps", bufs=4, space="PSUM") as hpsp,
            tc.tile_pool(name="hp", bufs=8) as hp_pool,
            tc.tile_pool(name="hpm8", bufs=4) as hpm8_pool,
            tc.tile_pool(name="ops", bufs=4, space="PSUM") as outps_pool,
            tc.tile_pool(name="osb", bufs=4) as osb_pool,
        ):
            # ---- constants / weights preload ----
            b1_sb = consts.tile([128, N_EXP * MC], F32, name="b1_sb")
            alpha_sb = consts.tile([128, N_ROUTED * MC], F32, name="alpha_sb")
            b2s_sb = consts.tile([128, OC], F32, name="b2s_sb")
            ones_sb = consts.tile([1, 128], BF16, name="ones_sb")
            b1row_sb = consts.tile([1, N_EXP * HID], BF16, name="b1row_sb")
            ones_row = consts.tile([1, TILE_N], BF16, name="ones_row")

            nc.sync.dma_start(b1_sb[:], b1_d[:])
            nc.sync.dma_start(alpha_sb[:], alpha_d[:])
            nc.sync.dma_start(b2s_sb[:], b2s_d[:])
            nc.sync.dma_start(b1row_sb[:], b1r_d[:])
            nc.vector.memset(ones_sb[:], 1.0)
            nc.vector.memset(ones_row[:], 1.0)

            # Per-expert weight tiles, loaded in first-use order. Tile-granular
            # deps let tile 0's matmuls start after the first expert arrives.
            # tile 0 runs routed-first (fp8 weights + x8 are the smallest
            # startup DMAs), later tiles shared-first; weights load in
            # first-use order
            eorder = []
            for _, segments in tiles:
                for _, _, ee in segments:
                    for e in list(ee) + [N_ROUTED + s for s in range(N_SHARED)]:
                        if e not in eorder:
                            eorder.append(e)
                if len(eorder) == N_EXP:
                    break
            for e in range(N_EXP):
                if e not in eorder:
                    eorder.append(e)

            # one big strided DMA per expert per matrix (queue-issue cost is
            # per-op, transfers run on the parallel DMA engines)
            w1t_v = w1t_d.rearrange("e (k p) h -> e p k h", p=128)
            w2t_v = w2t_d.rearrange("e (m p) o -> e p m o", p=128)
            w1t8_v = w1t8_d.rearrange("e (k p) h -> e p k h", p=128)
            w2t8_v = w2t8_d.rearrange("e (m p) o -> e p m o", p=128)
            w1_sb = {}
            w2_sb = {}

            def load_expert_w1(e):
                if e < N_ROUTED:
                    w1_sb[e] = consts.tile([128, KC, HID], FP8, name=f"w1e{e}")
                    nc.sync.dma_start(w1_sb[e][:], w1t8_v[e])
                else:
                    w1_sb[e] = consts.tile([128, KC, HID], BF16, name=f"w1e{e}")
                    nc.sync.dma_start(w1_sb[e][:], w1t_v[e - N_ROUTED])

            def load_expert_w2(e):
                if e < N_ROUTED:
                    w2_sb[e] = consts.tile([128, MC, OUT_F], FP8, name=f"w2e{e}")
                    nc.sync.dma_start(w2_sb[e][:], w2t8_v[e])
                else:
                    w2_sb[e] = consts.tile([128, MC, OUT_F], BF16, name=f"w2e{e}")
                    nc.sync.dma_start(w2_sb[e][:], w2t_v[e - N_ROUTED])

            def load_expert_weights(e):
                load_expert_w1(e)
                load_expert_w2(e)

            # mm1 weights of tile 0's first expert only — its mm2 weights
            # queue behind tile 0's inputs so the first matmul starts sooner
            load_expert_w1(eorder[0])

            # Warm the PE HAM clock gate (cold = 1.2 GHz until ~3.4us of
            # sustained activity) and the ScalarE activation table while the
            # first input DMAs are in flight.
            warm_sb = consts.tile([128, 1], F32, name="warm_sb")
            for _ in range(12):
                warm_ps = hpsp.tile([128, TILE_N], F32, name="warm_ps", tag="hps")
                nc.tensor.matmul(
                    warm_ps[:, :], ones_sb[:, :], ones_row[0:1, :],
                    start=True, stop=True,
                )
            nc.scalar.activation(
                warm_sb[:, :], b1_sb[:, 0:1],
                mybir.ActivationFunctionType.Sigmoid
                if SILU_DECOMP
                else mybir.ActivationFunctionType.Silu,
            )

            # ---- main loop over atom tiles ----
            xT_v = xT_d.rearrange("(k p) a -> p k a", p=128)
            x8_v = x8_d.rearrange("(k p) a -> p k a", p=128)
            outT_v = outT_d.rearrange("(c p) a -> p c a", p=128)
            a0 = 0
            for t, (n, segments) in enumerate(tiles):
                # routed-first needs one full-width PSUM start group, so it
                # requires a single-segment tile; only used for tile 0
                routed_first = t == 0 and len(segments) == 1
                x_sb = xp.tile([128, KC, TILE_N], BF16, name="x_sb", tag="x")
                x8_sb = x8p.tile([128, KC, TILE_N], FP8, name="x8_sb", tag="x8")
                w6row = w6p.tile([1, N_ROUTED, TILE_N], BF16, name="w6row", tag="w6r")
                if routed_first:
                    nc.sync.dma_start(x8_sb[:, :, :n], x8_v[:, :, a0 : a0 + n])
                    nc.sync.dma_start(w6row[0:1, :, :n], w6_d[:, a0 : a0 + n])
                    if t == 0:
                        load_expert_w2(eorder[0])
                    nc.sync.dma_start(x_sb[:, :, :n], xT_v[:, :, a0 : a0 + n])
                else:
                    nc.sync.dma_start(x_sb[:, :, :n], xT_v[:, :, a0 : a0 + n])
                    nc.sync.dma_start(x8_sb[:, :, :n], x8_v[:, :, a0 : a0 + n])
                    nc.sync.dma_start(w6row[0:1, :, :n], w6_d[:, a0 : a0 + n])
                    if t == 0:
                        load_expert_w2(eorder[0])

                if t == 0:
                    # stream the remaining experts' weights behind tile 0's
                    # inputs; tile 0's compute covers the transfer time
                    for e in eorder[1:]:
                        load_expert_weights(e)

                # per-atom gates broadcast across 128 partitions (GPSIMD,
                # keeps PE free); one broadcast per (segment, expert)
                wsb = {}
                for gi, (off, sn, ee) in enumerate(segments):
                    for e in ee:
                        wsb_e = wbcp.tile(
                            [128, TILE_N], BF16, name="wsb", tag="wbc"
                        )
                        nc.gpsimd.partition_broadcast(
                            wsb_e[:, :sn], w6row[0:1, e, off : off + sn]
                        )
                        wsb[gi, e] = wsb_e

                # output accumulators (PSUM values carry a x16 scale)
                outps = [
                    outps_pool.tile([128, TILE_N], F32, name="ops", tag="ops")
                    for _ in range(OC)
                ]

                def silu_to(h_ap, hps_ap, e, m, scale):
                    w = h_ap.shape[-1]
                    if SILU_DECOMP:
                        boff = e * HID + m * 128
                        nc.tensor.matmul(
                            hps_ap,
                            b1row_sb[0:1, boff : boff + 128],
                            ones_row[0:1, :w],
                            start=False, stop=True,
                        )
                        s_sb = hp_pool.tile(
                            [128, TILE_N], BF16, name="s_sb", tag="s"
                        )
                        nc.scalar.activation(
                            s_sb[:, :w], hps_ap,
                            mybir.ActivationFunctionType.Sigmoid,
                            scale=scale,
                        )
                        nc.vector.scalar_tensor_tensor(
                            h_ap, hps_ap, scale, s_sb[:, :w],
                            mybir.AluOpType.mult, mybir.AluOpType.mult,
                        )
                    else:
                        nc.scalar.activation(
                            h_ap, hps_ap,
                            mybir.ActivationFunctionType.Silu,
                            bias=b1_sb[:, e * MC + m : e * MC + m + 1],
                            scale=scale,
                        )

                def emit_routed(seg_start, stop_last):
                    # fp8 e4m3 DoubleRow experts, per column segment. With
                    # seg_start, the first mm2 group of each segment clears
                    # its PSUM subrange (per-element has_written).
                    for gi, (off, sn, ee) in enumerate(segments):
                        last_seg = gi == len(segments) - 1
                        for ei, e in enumerate(ee):
                            for kk in range(MC // 2):  # m-chunk pairs
                                hpm8 = hpm8_pool.tile(
                                    [128, 2, TILE_N], FP8,
                                    name="hpm8", tag="hpm8"
                                )
                                for j in range(2):
                                    m = 2 * kk + j
                                    hps = hpsp.tile(
                                        [128, TILE_N], F32,
                                        name="hps", tag="hps"
                                    )
                                    for k in range(KC // 2):
                                        nc.tensor.matmul(
                                            hps[:, :sn],
                                            w1_sb[e][:, 2 * k : 2 * k + 2,
                                                     m * 128 : (m + 1) * 128],
                                            x8_sb[:, 2 * k : 2 * k + 2,
                                                  off : off + sn],
                                            start=(k == 0),
                                            stop=(k == KC // 2 - 1
                                                  and not SILU_DECOMP),
                                            perf_mode=DR,
                                        )
                                    h_sb = hp_pool.tile(
                                        [128, TILE_N], BF16,
                                        name="h_sb", tag="h"
                                    )
                                    silu_to(
                                        h_sb[:, :sn], hps[:, :sn],
                                        e, m, 1.0 / WS,
                                    )
                                    ac = e * MC + m
                                    nc.vector.scalar_tensor_tensor(
                                        hpm8[:, j, :sn],
                                        h_sb[:, :sn],
                                        alpha_sb[:, ac : ac + 1],
                                        wsb[gi, e][:, :sn],
                                        mybir.AluOpType.add,
                                        mybir.AluOpType.mult,
                                    )
                                first = seg_start and ei == 0 and kk == 0
                                last = (stop_last and last_seg
                                        and ei == len(ee) - 1
                                        and kk == MC // 2 - 1)
                                for c in range(OC):
                                    nc.tensor.matmul(
                                        outps[c][:, off : off + sn],
                                        w2_sb[e][:, 2 * kk : 2 * kk + 2,
                                                 c * 128 : (c + 1) * 128],
                                        hpm8[:, :, :sn],
                                        start=first,
                                        stop=last,
                                        perf_mode=DR,
                                    )

                def emit_shared(start, stop_last):
                    # bf16 experts over the full tile (W2 carries the x16
                    # scale)
                    for si in range(N_SHARED):
                        e = N_ROUTED + si
                        for m in range(MC):
                            hps = hpsp.tile(
                                [128, TILE_N], F32, name="hps", tag="hps"
                            )
                            for k in range(KC):
                                nc.tensor.matmul(
                                    hps[:, :n],
                                    w1_sb[e][:, k, m * 128 : (m + 1) * 128],
                                    x_sb[:, k, :n],
                                    start=(k == 0),
                                    stop=(k == KC - 1 and not SILU_DECOMP),
                                )
                            h_sb = hp_pool.tile(
                                [128, TILE_N], BF16, name="h_sb", tag="h"
                            )
                            silu_to(h_sb[:, :n], hps[:, :n], e, m, 1.0)
                            first = start and si == 0 and m == 0
                            last = (stop_last and si == N_SHARED - 1
                                    and m == MC - 1)
                            for c in range(OC):
                                nc.tensor.matmul(
                                    outps[c][:, :n],
                                    w2_sb[e][:, m, c * 128 : (c + 1) * 128],
                                    h_sb[:, :n],
                                    start=first,
                                    stop=last,
                                )

                if routed_first:
                    emit_routed(seg_start=True, stop_last=False)
                    emit_shared(start=False, stop_last=True)
                else:
                    emit_shared(start=True, stop_last=False)
                    emit_routed(seg_start=False, stop_last=True)

                # psum -> sbuf (+ shared-expert b2, x16) on DVE: keeps the out
                # path off the in-order ScalarE queue, which lags on silus.
                # Per-c copy + DMA so c=0 streams out while c=1 copies.
                osb = osb_pool.tile([128, OC, TILE_N], F32, name="osb", tag="osb")
                for c in range(OC):
                    nc.vector.tensor_scalar_add(
                        osb[:, c, :n], outps[c][:, :n], b2s_sb[:, c : c + 1]
                    )
                    nc.sync.dma_start(
                        outT_v[:, c, a0 : a0 + n], osb[:, c, :n]
                    )
                a0 += n

    nc.compile()
    return nc


def _alpha_pack(rW2, rb2):
    """alpha_e = min-norm solution of W2_e @ alpha = b2_e, packed per-chunk."""
    alphas = []
    for e in range(N_ROUTED):
        a, *_ = np.linalg.lstsq(rW2[e].astype(np.float64), rb2[e].astype(np.float64))
        alphas.append(a)
    al = np.stack(alphas).astype(np.float32)  # [6, HID]
    return np.ascontiguousarray(
        al.reshape(N_ROUTED, MC, 128).transpose(2, 0, 1).reshape(128, N_ROUTED * MC)
    )


def _prep_host(inputs):
    feats = np.asarray(inputs["features"], dtype=np.float32)
    species = np.asarray(inputs["species_idx"]).astype(np.int64)
    emb = np.asarray(inputs["emb"], dtype=np.float32)
    Wr = np.asarray(inputs["W_router"], dtype=np.float32)
    rW1 = np.asarray(inputs["rW1"], dtype=np.float32)
    rb1 = np.asarray(inputs["rb1"], dtype=np.float32)
    rW2 = np.asarray(inputs["rW2"], dtype=np.float32)
    rb2 = np.asarray(inputs["rb2"], dtype=np.float32)
    sW1 = np.asarray(inputs["sW1"], dtype=np.float32)
    sb1 = np.asarray(inputs["sb1"], dtype=np.float32)
    sW2 = np.asarray(inputs["sW2"], dtype=np.float32)
    sb2 = np.asarray(inputs["sb2"], dtype=np.float32)

    wt_table, top2 = _router_table(emb, Wr)
    idx_cores, tiles = _plan_sharding(species, top2)
    nl = idx_cores.shape[1]
    w_atoms = wt_table[species]  # [n, 6] f32

    b1 = np.concatenate([rb1, sb1], axis=0)  # [8, HID]
    # decomp-path row bias: routed entries carry the x16 PSUM scale
    b1r = b1.copy()
    b1r[:N_ROUTED] *= WS

    shared = {
        "w1t": np.ascontiguousarray(sW1.transpose(0, 2, 1)).astype(BF16_NP),
        "w2t": np.ascontiguousarray(sW2.transpose(0, 2, 1) * WS).astype(BF16_NP),
        "w1t8": np.ascontiguousarray(rW1.transpose(0, 2, 1) * WS).astype(FP8_NP),
        "w2t8": np.ascontiguousarray(rW2.transpose(0, 2, 1) * WS).astype(FP8_NP),
        "b1": np.ascontiguousarray(
            b1.reshape(N_EXP, MC, 128).transpose(2, 0, 1).reshape(128, N_EXP * MC)
        ),
        "b1r": b1r.reshape(1, N_EXP * HID).astype(BF16_NP),
        "alpha": _alpha_pack(rW2, rb2),
        "b2s": np.ascontiguousarray((sb2.sum(axis=0) * WS).reshape(OC, 128).T),
    }

    in_maps = []
    for c in range(N_CORES):
        idx = idx_cores[c]
        valid = idx >= 0
        iv = idx[valid]
        fT = np.ascontiguousarray(feats[iv].T)
        xT = np.zeros((IN_F, nl), dtype=BF16_NP)
        xT[:, valid] = fT.astype(BF16_NP)
        x8 = np.zeros((IN_F, nl), dtype=FP8_NP)
        x8[:, valid] = fT.astype(FP8_NP)
        w6 = np.zeros((N_ROUTED, nl), dtype=BF16_NP)
        w6[:, valid] = np.ascontiguousarray(w_atoms[iv].T).astype(BF16_NP)
        in_maps.append({"xT": xT, "x8": x8, "w6": w6, **shared})
    return in_maps, idx_cores, tiles, nl, feats.shape[0]


_PROGRAM_CACHE = {}


def _get_program(nl, tiles):
    key = (nl, tuple(tiles))
    if key not in _PROGRAM_CACHE:
        _PROGRAM_CACHE[key] = _build_program(nl, tiles)
    return _PROGRAM_CACHE[key]


# Set TRACE=True (e.g. from a test harness) to capture a neuron-profile trace;
# the full BassKernelResults of the last run is kept in LAST_RESULTS.
TRACE = False
LAST_RESULTS = None


def kernel(**inputs):
    global LAST_RESULTS
    in_maps, idx_cores, tiles, nl, n_atoms = _prep_host(inputs)
    nc = _get_program(nl, tiles)
    res = run_bass_kernel_spmd(nc, in_maps, list(range(N_CORES)), trace=TRACE)
    LAST_RESULTS = res
    out = np.zeros((n_atoms, OUT_F), dtype=np.float32)
    inv = np.float32(1.0 / WS)
    for c in range(N_CORES):
        idx = idx_cores[c]
        valid = idx >= 0
        outT = res.results[c]["outT"]  # [OUT_F, nl] f32, x16 scale
        out[idx[valid]] = outT[:, valid].T * inv
    return out
